# revision 1
# baseline (speedup 1.0000x reference)
"""Causal self-attention (weight-modulated) Trainium2 kernel, 8-core SPMD.

Reference semantics (B=2, T=2048, C=512, 8 heads, hd=64):
    v0  = x @ Wv.T + bv
    v   = v0 * w[:, :, None]            # w = weight[:, :, 0]
    att = softmax(mask((v0h @ v0h^T) * w[key] / sqrt(hd)))
    y   = att @ vh
    out = y @ Wp.T + bp

Sharding: core = (b, p) with b = batch, p = query-quarter. Each core
computes 512 contiguous query rows against all keys of its batch.
Keys are host-permuted so the causal diagonal 512-block sits at key
slot 0 for every core; the program is identical across cores (SPMD)
and per-core differences live entirely in the input data:
  - kxT   [C, T]   x^T with permuted+padded key columns
  - kbvec [T, 1]   additive exp-bias: 0 real keys, -1e30 padding
  - w8vec [T, 1]   w_perm / sqrt(hd) (exp scale; also folds wl)
  - wvec  [T, 1]   w_perm (value scaling wr)
Because wl == wr == w, the "keys" of the score matmul are just the
value projection v0 (+bv) with no w applied, and w is applied once in
the exp scale (score layout is [key, query], so w[key] is a
per-partition scalar). Queries are slot 0 of the same v0^T tensor.
The softmax denominator comes for free as a 65th ones-column in the
AV matmul's stationary operand.
"""

import ml_dtypes
import numpy as np

B, T, C = 2, 2048, 512
NH, HD = 8, 64
P = 128
QB = 512                # query rows per core
NKB = 4                 # key blocks of 512
NSB = 16                # key sub-blocks of 128
NEG = -1.0e30

_cache = {}


def _split_multi_waits(nc, mybir):
    """Walrus in this container encodes at most ONE sync wait (and one
    update) per instruction; Tile's sem assignment emits several. Hoist
    excess waits onto single-wait NOPs placed just before the
    instruction on the same engine (sequencer semantics are identical:
    the engine blocks on each wait, then issues the instruction), and
    excess updates of non-DMA instructions onto NOPs just after."""
    dma_ops = {"DMACopy", "DMATranspose", "TensorCopy"}
    for f in nc.m.functions:
        for bb in f.blocks:
            new = []
            changed = False
            for inst in bb.instructions:
                si = inst.sync_info
                waits = list(si.on_wait or []) if si is not None else []
                ups = list(si.on_update or []) if si is not None else []
                is_dma = inst.concise_opcode() in dma_ops if hasattr(
                    inst, "concise_opcode") else False
                post = []
                if si is not None and len(waits) > 1:
                    for w in waits[:-1]:
                        nop = mybir.InstNoOp(
                            name=nc.get_next_instruction_name(),
                            sync_info=mybir.SyncInfo(on_wait=[w], on_update=[]),
                            bass_nofuse=True,
                            engine=inst.engine,
                        )
                        nc.register_instruction(nop, overwrite=True)
                        new.append(nop)
                    waits = waits[-1:]
                    inst.sync_info = mybir.SyncInfo(on_wait=waits, on_update=ups)
                    changed = True
                if si is not None and len(ups) > 1 and not is_dma:
                    for u in ups[1:]:
                        nop = mybir.InstNoOp(
                            name=nc.get_next_instruction_name(),
                            sync_info=mybir.SyncInfo(on_wait=[], on_update=[u]),
                            bass_nofuse=True,
                            engine=inst.engine,
                        )
                        nc.register_instruction(nop, overwrite=True)
                        post.append(nop)
                    inst.sync_info = mybir.SyncInfo(
                        on_wait=waits, on_update=ups[:1])
                    changed = True
                new.append(inst)
                new.extend(post)
            if changed:
                bb.instructions = new


def _trineg_const():
    # trineg[s, k, u*512 + t] = 0 where query t (local) may see key 128k+s
    # (t >= 128k+s), else NEG; repeated twice along the free axis so one
    # DVE op can mask a head-pair's [128, 1024] score group.
    s = np.arange(P)[:, None]
    t = np.arange(QB)[None, :]
    out = np.empty((P, NKB, 2 * QB), np.float32)
    for k in range(NKB):
        blk = np.where(t >= P * k + s, 0.0, NEG)
        out[:, k, 0:QB] = blk
        out[:, k, QB:2 * QB] = blk
    return out.reshape(P, NKB * 2 * QB).astype(ml_dtypes.bfloat16)


def _build_nc(with_bias):
    import concourse.bass as bass
    import concourse.mybir as mybir

    from concourse.tile import TileContext
    f32 = mybir.dt.float32
    f32r = mybir.dt.float32r
    bf16 = mybir.dt.bfloat16
    AF = mybir.ActivationFunctionType
    ALU = mybir.AluOpType

    nc = bass.Bass()

    kxT = nc.dram_tensor("kxT", [C, T], bf16, kind="ExternalInput")
    wvt = nc.dram_tensor("wvt", [C, C], bf16, kind="ExternalInput")
    wpt = nc.dram_tensor("wpt", [C, C], bf16, kind="ExternalInput")
    bvp = nc.dram_tensor("bvp", [C, 1], f32, kind="ExternalInput")
    bvr = nc.dram_tensor("bvr", [1, C], bf16, kind="ExternalInput")
    bpp = nc.dram_tensor("bpp", [C, 1], f32, kind="ExternalInput")
    wvec = nc.dram_tensor("wvec", [T, 1], f32, kind="ExternalInput")
    w8vec = nc.dram_tensor("w8vec", [T, 1], f32, kind="ExternalInput")
    kbvec = nc.dram_tensor("kbvec", [T, 1], f32, kind="ExternalInput")
    onesr = nc.dram_tensor("onesr", [1, P], bf16,
                           kind="ExternalInput")
    onesrr = nc.dram_tensor("onesrr", [1, HD], f32r, kind="ExternalInput")
    onescol = nc.dram_tensor("onescol", [P, NH], bf16, kind="ExternalInput")
    outT = nc.dram_tensor("outT", [C, QB], f32, kind="ExternalOutput")

    trineg_d = nc.inline_tensor(_trineg_const(), name="trineg")

    def r(ap):
        return ap

    with TileContext(nc) as tc:
        with (
            tc.tile_pool(name="persist", bufs=1) as pp,
            tc.tile_pool(name="stream", bufs=3) as sp,
            tc.tile_pool(name="psum", bufs=2, space="PSUM") as qq,
        ):
            # ---- persistent SBUF tensors ----
            kx_sb = [pp.tile([P, T], bf16, tag=f"kx{i}", name=f"kx{i}") for i in range(4)]
            wvt_sb = [pp.tile([P, C], bf16, tag=f"wvt{i}", name=f"wvt{i}") for i in range(4)]
            wpt_sb = [pp.tile([P, C], bf16, tag=f"wpt{i}", name=f"wpt{i}") for i in range(4)]
            vT_sb = [pp.tile([P, T], bf16, tag=f"vT{i}", name=f"vT{i}") for i in range(4)]
            # 8*(64+1) value columns + 63-col pad so every head has a
            # [128, 128] lhsT window (M=128 enables fast weight load; the
            # extra columns only feed unread PSUM rows 65..127)
            va_sb = [pp.tile([P, NH * (HD + 1) + HD - 1], bf16,
                             tag=f"va{i}", name=f"va{i}")
                     for i in range(NSB)]
            y_sb = [pp.tile([P, QB], bf16, tag=f"y{i}", name=f"ySB{i}") for i in range(4)]
            tri_sb = pp.tile([P, NKB * 2 * QB], bf16, tag="tri")
            bvp_sb = pp.tile([P, C // P], f32, tag="bvp")
            bvr_sb = pp.tile([1, C], bf16, tag="bvr")
            bpp_sb = pp.tile([P, C // P], f32, tag="bpp")
            wv_sb = pp.tile([P, NSB], f32, tag="wv")
            w8_sb = pp.tile([P, NSB], f32, tag="w8")
            kb_sb = pp.tile([P, NSB], f32, tag="kb")
            ones_sb = pp.tile([1, P], bf16, tag="ones")
            onesr_sb = pp.tile([1, HD], f32r, tag="onesr")
            onesc_sb = pp.tile([P, NH], bf16, tag="onesc")

            for i in range(4):
                nc.sync.dma_start(out=wvt_sb[i][:], in_=wvt[i * P:(i + 1) * P, :])
            nc.sync.dma_start(
                out=bvp_sb[:], in_=bvp.rearrange("(n p) o -> p (n o)", p=P))
            nc.sync.dma_start(out=bvr_sb[:], in_=bvr[:])
            nc.sync.dma_start(
                out=wv_sb[:], in_=wvec.rearrange("(n p) o -> p (n o)", p=P))
            nc.sync.dma_start(
                out=w8_sb[:], in_=w8vec.rearrange("(n p) o -> p (n o)", p=P))
            nc.sync.dma_start(
                out=kb_sb[:], in_=kbvec.rearrange("(n p) o -> p (n o)", p=P))
            nc.sync.dma_start(out=ones_sb[:], in_=onesr[:])
            nc.sync.dma_start(out=onesr_sb[:], in_=onesrr[:])
            nc.sync.dma_start(out=onesc_sb[:], in_=onescol[:])
            for j2 in range(2 * NKB):
                for i in range(4):
                    nc.sync.dma_start(
                        out=kx_sb[i][:, j2 * 256:(j2 + 1) * 256],
                        in_=kxT[i * P:(i + 1) * P, j2 * 256:(j2 + 1) * 256])
            nc.sync.dma_start(out=tri_sb[:], in_=trineg_d[:])
            for i in range(4):
                nc.sync.dma_start(out=wpt_sb[i][:], in_=wpt[i * P:(i + 1) * P, :])
            nc.sync.dma_start(
                out=bpp_sb[:], in_=bpp.rearrange("(n p) o -> p (n o)", p=P))

            # ---- phase A (j-outer for DMA overlap):
            # vT = (x @ Wv.T + bv)^T [c, s]; v_aug = (v0 + bv) * w[s] ----
            for j in range(NKB):
                for i in range(4):        # c' partition block of vT
                    ps = qq.tile([P, QB], f32, tag="vps", name="vps")
                    for k in range(4):    # contraction block
                        nc.tensor.matmul(
                            ps[:],
                            r(wvt_sb[k][:, i * P:(i + 1) * P]),
                            r(kx_sb[k][:, j * QB:(j + 1) * QB]),
                            start=(k == 0), stop=(k == 3),
                        )
                    nc.vector.tensor_scalar_add(
                        vT_sb[i][:, j * QB:(j + 1) * QB], ps[:], bvp_sb[:, i:i + 1])
                for sb in range(4 * j, 4 * j + 4):
                    ps = qq.tile([P, C], f32, tag="vps", name="vps")
                    for k in range(4):
                        nc.tensor.matmul(
                            ps[:],
                            r(kx_sb[k][:, sb * P:(sb + 1) * P]),
                            r(wvt_sb[k][:]),
                            start=(k == 0),
                            stop=(k == 3 and not with_bias),
                        )
                    if with_bias:
                        # += ones[s] x bv (K=1 matmul, free-axis bias)
                        nc.tensor.matmul(
                            ps[:], r(ones_sb[:]), r(bvr_sb[:]),
                            start=False, stop=True)
                    nc.vector.memset(
                        va_sb[sb][:, NH * (HD + 1):NH * (HD + 1) + HD - 1], 0.0)
                    va3 = va_sb[sb][:, 0:NH * (HD + 1)].rearrange(
                        "p (h d) -> p h d", d=HD + 1)
                    nc.vector.tensor_scalar_mul(
                        va3[:, :, 0:HD],
                        ps[:].rearrange("p (h d) -> p h d", d=HD),
                        wv_sb[:, sb:sb + 1],
                    )
                    nc.vector.tensor_copy(
                        va3[:, :, HD:HD + 1],
                        onesc_sb[:].rearrange("p (h o) -> p h o", o=1))

            # ---- phase B: head pairs; even head in PE rows 0-63, odd in
            # 64-127 -> their QK matmuls run concurrently via row tiling.
            # The pair's scores share one [128, 1024] PSUM group so a
            # single DVE-mask / ACT-exp op covers both heads. ----
            for hp in range(NH // 2):
                ti = hp // 2
                yps2 = [qq.tile([P, QB], f32, tag="y", name="yps") for _ in range(2)]
                for sb in range(NSB):
                    spair = qq.tile([P, 2 * QB], f32, tag="S", name="spair")
                    for u in range(2):
                        h = 2 * hp + u
                        po = (h % 2) * HD
                        nc.tensor.matmul(
                            spair[:, u * QB:(u + 1) * QB],
                            vT_sb[h // 2][po:po + HD, sb * P:(sb + 1) * P],
                            vT_sb[h // 2][po:po + HD, 0:QB],
                            start=True, stop=True,
                        )
                    e = sp.tile([P, 2 * QB], bf16, tag="e", name="e", bufs=6)
                    if sb < 4:
                        s2 = sp.tile([P, 2 * QB], f32, tag="s2", name="s2", bufs=3)
                        nc.vector.scalar_tensor_tensor(
                            s2[:], spair[:], w8_sb[:, sb:sb + 1],
                            tri_sb[:, sb * 2 * QB:(sb + 1) * 2 * QB],
                            ALU.mult, ALU.add,
                        )
                        nc.scalar.activation(
                            e[:], s2[:], AF.Exp,
                            bias=kb_sb[:, sb:sb + 1], scale=1.0)
                    else:
                        nc.scalar.activation(
                            e[:], spair[:], AF.Exp,
                            bias=kb_sb[:, sb:sb + 1],
                            scale=w8_sb[:, sb:sb + 1])
                    for u in range(2):
                        h = 2 * hp + u
                        nc.tensor.matmul(
                            yps2[u][:, :],
                            va_sb[sb][:, h * (HD + 1):h * (HD + 1) + P],
                            e[:, u * QB:(u + 1) * QB],
                            start=(sb == 0), stop=(sb == NSB - 1),
                        )
                # per-head tails: 1/denom = exp(-ln(d)) on ACT (cost is
                # free-dim driven, so the [1, 512] row is cheap), then one
                # partition-broadcast DVE multiply to normalize
                for u in range(2):
                    h = 2 * hp + u
                    ti, po = h // 2, (h % 2) * HD
                    yps = yps2[u]
                    ld = sp.tile([1, QB], f32, tag="ld", name="ld")
                    nc.scalar.activation(ld[:], yps[HD:HD + 1, :], AF.Ln)
                    rec = sp.tile([1, QB], f32r, tag="rec", name="rec")
                    nc.scalar.activation(rec[:], ld[:], AF.Exp, scale=-1.0)
                    dps = qq.tile([P, QB], f32, tag="vps", name="dps")
                    nc.tensor.matmul(
                        dps[0:HD, :], r(onesr_sb[:]), r(rec[:]),
                        start=True, stop=True,
                    )
                    nc.vector.tensor_copy(
                        y_sb[ti][po:po + HD, :], yps[0:HD, :])
                    nc.vector.tensor_mul(
                        y_sb[ti][po:po + HD, :],
                        y_sb[ti][po:po + HD, :], dps[0:HD, :])

            # ---- phase C: out^T = Wp @ y^T + bp ----
            for i in range(4):
                ops = qq.tile([P, QB], f32, tag="vps", name="vps")
                for k in range(4):
                    nc.tensor.matmul(
                        ops[:],
                        r(wpt_sb[k][:, i * P:(i + 1) * P]),
                        r(y_sb[k][:]),
                        start=(k == 0), stop=(k == 3),
                    )
                ot = sp.tile([P, QB], f32, tag="ot", name="ot")
                nc.vector.tensor_scalar_add(ot[:], ops[:], bpp_sb[:, i:i + 1])
                nc.sync.dma_start(out=outT[i * P:(i + 1) * P, :], in_=ot[:])

    _split_multi_waits(nc, mybir)
    return nc


def _get_nc(with_bias=False):
    key = f"nc{int(with_bias)}"
    if key not in _cache:
        _cache[key] = _build_nc(with_bias)
    return _cache[key]


def _make_in_maps(x, weight, Wv, bv, Wp, bp, state):
    x = np.asarray(x, np.float32)
    w = np.asarray(weight, np.float32)[:, :, 0]
    if not int(np.asarray(state)):
        w = np.ones_like(w)
    WvT = np.ascontiguousarray(np.asarray(Wv, np.float32).T)
    WpT = np.ascontiguousarray(np.asarray(Wp, np.float32).T)
    bv = np.asarray(bv, np.float32)
    bp = np.asarray(bp, np.float32)
    scale = 1.0 / np.sqrt(HD)

    in_maps = []
    for core in range(8):
        b, p = core // 4, core % 4
        nreal = QB * (p + 1)
        perm = np.concatenate(
            [np.arange(QB * p, QB * (p + 1)), np.arange(0, QB * p)])
        kx = np.zeros((T, C), np.float32)
        kx[:nreal] = x[b][perm]
        wp_ = np.zeros((T,), np.float32)
        wp_[:nreal] = w[b][perm]
        kb = np.zeros((T, 1), np.float32)
        kb[nreal:] = NEG
        in_maps.append({
            "kxT": np.ascontiguousarray(kx.T).astype(ml_dtypes.bfloat16),
            "wvt": WvT.astype(ml_dtypes.bfloat16),
            "wpt": WpT.astype(ml_dtypes.bfloat16),
            "bvp": bv.reshape(C, 1),
            "bvr": bv.reshape(1, C).astype(ml_dtypes.bfloat16),
            "bpp": bp.reshape(C, 1),
            "wvec": wp_.reshape(T, 1).copy(),
            "w8vec": (wp_ * scale).reshape(T, 1).copy(),
            "kbvec": kb,
            "onesr": np.ones((1, P), ml_dtypes.bfloat16),
            "onesrr": np.ones((1, HD), np.float32),
            "onescol": np.ones((P, NH), ml_dtypes.bfloat16),
        })
    return in_maps


def _gather(results, x):
    out = np.empty((B, T, C), np.float32)
    for core in range(8):
        b, p = core // 4, core % 4
        out[b, QB * p:QB * (p + 1), :] = results[core]["outT"].T
    return out


def _run(in_maps, with_bias=False, **kw):
    from concourse.bass_utils import run_bass_kernel_spmd
    return run_bass_kernel_spmd(
        _get_nc(with_bias), in_maps, list(range(8)), **kw)


def kernel(x, weight, Wv, bv, Wp, bp, state):
    in_maps = _make_in_maps(x, weight, Wv, bv, Wp, bp, state)
    res = _run(in_maps, with_bias=bool(np.any(np.asarray(bv))))
    return _gather(res.results, x)



# revision 6
# speedup vs baseline: 1.2885x; 1.2885x over previous
"""Causal self-attention (weight-modulated) Trainium2 kernel, 8-core SPMD.

Reference semantics (B=2, T=2048, C=512, 8 heads, hd=64):
    v0  = x @ Wv.T + bv
    v   = v0 * w[:, :, None]            # w = weight[:, :, 0]
    att = softmax(mask((v0h @ v0h^T) * w[key] / sqrt(hd)))
    y   = att @ vh
    out = y @ Wp.T + bp

Sharding: core = (b, p) with b = batch, p = query-quarter. Each core
computes 512 contiguous query rows against all keys of its batch.
Keys are host-permuted so the causal diagonal 512-block sits at key
slot 0 for every core; the program is identical across cores (SPMD).

Dataflow per core:
  A: vT = (x @ Wv.T + bv)^T            [c, keyslot]  (bf16, 4x matmul)
  T: va = transpose(vT) * w[key]       [keyslot, (h: v_h | real)]
     via PE transposes (cost = 128 rows each; much cheaper than a
     second matmul pass).  "real" = 1 for real keys, 0 for padding:
     it feeds the softmax-denominator column of the AV matmul, so no
     -inf bias is needed for padding (padded x columns are zero).
  B (per head pair): scores = vT^T vT into PSUM [keyslot, 2*512],
     trimmed causally on the diagonal 512-block; e = exp(w8 * s) on
     ACT; in-diagonal triangular mask applied post-exp as a 0/1
     multiply on DVE (cheap 2-byte op); AV with e as the stationary
     operand: yps[q, h: y_h | denom] += e_tile^T @ [va_h | real]
     (N=65 per matmul -- half the streamed rows of the [hd,q] form),
     then normalize with DVE reciprocal + per-partition scalars.
  C: y2 [q, c] -> PE transpose -> y^T [c, q]; out^T = Wp @ y^T + bp.

Phase A/T work is interleaved into head-pair 0's slot loop so the
ACT engine gets exp work early and the PE never idles (p-state).
"""

import ml_dtypes
import numpy as np

B, T, C = 2, 2048, 512
NH, HD = 8, 64
P = 128
QB = 512                # query rows per core
NSB = 16                # key sub-blocks of 128
HW = HD + 1             # head window in va / yps: 64 values + denom
VAW = NH * HW           # 520 columns per key sub-block in va

_cache = {}


def _split_multi_waits(nc, mybir):
    """Walrus in this container encodes at most ONE sync wait (and one
    update) per instruction; Tile's sem assignment emits several. Hoist
    excess waits onto single-wait NOPs placed just before the
    instruction on the same engine (sequencer semantics are identical:
    the engine blocks on each wait, then issues the instruction), and
    excess updates of non-DMA instructions onto NOPs just after."""
    dma_ops = {"DMACopy", "DMATranspose", "TensorCopy"}
    for f in nc.m.functions:
        for bb in f.blocks:
            new = []
            changed = False
            for inst in bb.instructions:
                si = inst.sync_info
                waits = list(si.on_wait or []) if si is not None else []
                ups = list(si.on_update or []) if si is not None else []
                is_dma = inst.concise_opcode() in dma_ops if hasattr(
                    inst, "concise_opcode") else False
                post = []
                if si is not None and len(waits) > 1:
                    for w in waits[:-1]:
                        nop = mybir.InstNoOp(
                            name=nc.get_next_instruction_name(),
                            sync_info=mybir.SyncInfo(on_wait=[w], on_update=[]),
                            bass_nofuse=True,
                            engine=inst.engine,
                        )
                        nc.register_instruction(nop, overwrite=True)
                        new.append(nop)
                    waits = waits[-1:]
                    inst.sync_info = mybir.SyncInfo(on_wait=waits, on_update=ups)
                    changed = True
                if si is not None and len(ups) > 1 and not is_dma:
                    for u in ups[1:]:
                        nop = mybir.InstNoOp(
                            name=nc.get_next_instruction_name(),
                            sync_info=mybir.SyncInfo(on_wait=[], on_update=[u]),
                            bass_nofuse=True,
                            engine=inst.engine,
                        )
                        nc.register_instruction(nop, overwrite=True)
                        post.append(nop)
                    inst.sync_info = mybir.SyncInfo(
                        on_wait=waits, on_update=ups[:1])
                    changed = True
                new.append(inst)
                new.extend(post)
            if changed:
                bb.instructions = new


def _ctri_const():
    # [identity(128) | tri01(128)]: identity feeds PE transposes; tri01
    # is the in-diagonal causal mask: tri01[s, t] = 1 iff t >= s.
    s = np.arange(P)[:, None]
    t = np.arange(P)[None, :]
    out = np.empty((P, 2 * P), np.float32)
    out[:, 0:P] = (t == s)
    out[:, P:2 * P] = (t >= s)
    return out.astype(ml_dtypes.bfloat16)


def _build_nc():
    import concourse.bass as bass
    import concourse.mybir as mybir

    from concourse.tile import TileContext
    f32 = mybir.dt.float32
    bf16 = mybir.dt.bfloat16
    AF = mybir.ActivationFunctionType

    nc = bass.Bass()

    # vecs columns: 0-15 w (per key sub-block), 16-31 w/sqrt(hd),
    # 32-35 bv (c-block major), 36-39 bp
    kxd = nc.dram_tensor("kxd", [P, 4, T], bf16, kind="ExternalInput")
    wvtd = nc.dram_tensor("wvtd", [P, 4 * C], bf16, kind="ExternalInput")
    wptd = nc.dram_tensor("wptd", [P, 4 * C], bf16, kind="ExternalInput")
    vecsd = nc.dram_tensor("vecsd", [P, 40], f32, kind="ExternalInput")
    realzd = nc.dram_tensor("realzd", [P, P], bf16, kind="ExternalInput")
    outT = nc.dram_tensor("outT", [C, QB], f32, kind="ExternalOutput")

    ctri_d = nc.inline_tensor(_ctri_const(), name="ctri")

    with TileContext(nc) as tc:
        with (
            tc.tile_pool(name="persist", bufs=1) as pp,
            tc.tile_pool(name="stream", bufs=3) as sp,
            tc.tile_pool(name="psum", bufs=2, space="PSUM") as qq,
        ):
            # ---- persistent SBUF tensors ----
            kx = pp.tile([P, 4 * T], bf16, tag="kx")        # x^T, c-blk major
            wvt = pp.tile([P, 4 * C], bf16, tag="wvt")      # Wv^T, row-blk major
            wpt = pp.tile([P, 4 * C], bf16, tag="wpt")
            vT = pp.tile([P, 4 * T], bf16, tag="vT")        # v0^T, c-blk major
            va = pp.tile([P, NSB * VAW], bf16, tag="va")    # [slot, h: v|real]
            y2 = pp.tile([P, 4 * QB], bf16, tag="y2")       # [q, qb-major c]
            ysb = pp.tile([P, 4 * QB], bf16, tag="ysb")     # y^T, c-blk major
            vecs = pp.tile([P, 40], f32, tag="vecs")
            realz = pp.tile([P, P], bf16, tag="realz")
            ctri = pp.tile([P, 2 * P], bf16, tag="ctri")
            ident = ctri[:, 0:P]
            tri01 = ctri[:, P:2 * P]

            kx3 = kx[:].rearrange("p (k t) -> p k t", t=T)
            nc.sync.dma_start(out=wvt[:], in_=wvtd[:])
            nc.sync.dma_start(out=kx3[:, :, 0:QB], in_=kxd[:, :, 0:QB])
            nc.sync.dma_start(out=kx3[:, :, QB:2 * QB], in_=kxd[:, :, QB:2 * QB])
            nc.sync.dma_start(out=ctri[:], in_=ctri_d[:])
            nc.sync.dma_start(out=vecs[:], in_=vecsd[:])
            nc.sync.dma_start(out=realz[:], in_=realzd[:])
            nc.sync.dma_start(out=kx3[:, :, 2 * QB:3 * QB],
                              in_=kxd[:, :, 2 * QB:3 * QB])
            nc.sync.dma_start(out=kx3[:, :, 3 * QB:4 * QB],
                              in_=kxd[:, :, 3 * QB:4 * QB])
            nc.sync.dma_start(out=wpt[:], in_=wptd[:])

            # denominator indicator column: va[s, sb, h, 64] = real(sb,s)
            va4 = va[:].rearrange("p (s h w) -> p s h w", h=NH, w=HW)
            nc.vector.tensor_copy(
                va4[:, :, :, HD:HW].squeeze(3),
                realz[:].rearrange("p (s h) -> p s h", h=NH),
            )

            def emit_A(j, i):
                # vT[c-block i, key slots j*512:(j+1)*512]
                ps = qq.tile([P, 2 * QB], f32, tag="S", name="vps")
                for k in range(4):
                    nc.tensor.matmul(
                        ps[:, 0:QB],
                        wvt[:, k * C + i * P:k * C + (i + 1) * P],
                        kx[:, k * T + j * QB:k * T + (j + 1) * QB],
                        start=(k == 0), stop=(k == 3),
                    )
                nc.vector.tensor_scalar_add(
                    vT[:, i * T + j * QB:i * T + (j + 1) * QB],
                    ps[:, 0:QB], vecs[:, 32 + i:33 + i])

            def emit_T(sb):
                # va[sb] = transpose(vT[:, sb]) * w[key]
                tp = qq.tile([P, QB], bf16, tag="TP", name="tp")
                for i in range(4):
                    nc.tensor.transpose(
                        tp[:, i * P:(i + 1) * P],
                        vT[:, i * T + sb * P:i * T + (sb + 1) * P],
                        ident,
                    )
                nc.vector.tensor_scalar_mul(
                    va4[:, sb:sb + 1, :, 0:HD].squeeze(1),
                    tp[:].rearrange("p (h d) -> p h d", d=HD),
                    vecs[:, sb:sb + 1],
                )

            work = []
            for j in range(1, 4):
                for i in range(4):
                    work.append((emit_A, j, i))
                for sb in range(4 * j, 4 * j + 4):
                    work.append((emit_T, sb))

            emit_A(0, 0)
            emit_A(0, 1)
            emit_A(0, 2)
            emit_A(0, 3)
            for sb in range(4):
                emit_T(sb)

            # ---- phase B: head pairs, software-pipelined ----
            # QKE(hp) slot sb: scores for 128 keys x all later queries of
            # both heads, exp (per-key scale folds w and 1/sqrt(hd)),
            # 0/1 triangular mask on the diagonal square.  The 16 e tiles
            # of a pair stay alive (bufs=33) so AV can then run qb-major
            # with one complete PSUM accumulation group at a time (the PE
            # model corrupts interleaved open groups within a bank).
            es = [[] for _ in range(NH // 2)]

            def qke_slot(hp, sb):
                off = sb * P if sb < 4 else 0
                spair = qq.tile([P, 2 * QB], f32, tag="S", name="spair")
                sp3 = spair[:].rearrange("p (u t) -> p u t", t=QB)
                for u in range(2):
                    po = u * HD
                    nc.tensor.matmul(
                        sp3[:, u, off:QB],
                        vT[po:po + HD,
                           hp * T + sb * P:hp * T + (sb + 1) * P],
                        vT[po:po + HD, hp * T + off:hp * T + QB],
                        start=True, stop=True,
                    )
                e = sp.tile([P, 2 * QB], bf16, tag="e", name="e", bufs=33)
                e3 = e[:].rearrange("p (u t) -> p u t", t=QB)
                nc.scalar.activation(
                    e3[:, :, off:QB], sp3[:, :, off:QB], AF.Exp,
                    scale=vecs[:, 16 + sb:17 + sb])
                if sb < 4:
                    for u in range(2):
                        seg = e[:, u * QB + off:u * QB + off + P]
                        nc.vector.tensor_mul(seg, seg, tri01)
                es[hp].append(e)

            def av_group(hp, yps2, u, qb):
                h = 2 * hp + u
                sbs = [s for s in range(NSB) if s >= 4 or s <= qb]
                for n, sb in enumerate(sbs):
                    nc.tensor.matmul(
                        yps2[u][:, qb * HW:(qb + 1) * HW],
                        es[hp][sb][:, u * QB + qb * P:u * QB + (qb + 1) * P],
                        va[:, sb * VAW + h * HW:sb * VAW + (h + 1) * HW],
                        start=(n == 0), stop=(n == len(sbs) - 1),
                    )

            for sb in range(NSB):
                for _ in range(2):
                    if work:
                        fn, *args = work.pop(0)
                        fn(*args)
                qke_slot(0, sb)

            for hp in range(NH // 2):
                yps2 = [qq.tile([P, 4 * HW], f32, tag=f"Y{u}", name=f"yps{u}",
                                bufs=1) for u in range(2)]
                gi = 0
                for u in range(2):
                    for qb in range(4):
                        av_group(hp, yps2, u, qb)
                        if hp + 1 < NH // 2:
                            qke_slot(hp + 1, 2 * gi)
                            qke_slot(hp + 1, 2 * gi + 1)
                        gi += 1
                for u in range(2):
                    h = 2 * hp + u
                    yv = yps2[u][:].rearrange("p (q w) -> p q w", w=HW)
                    rec = sp.tile([P, 4], f32, tag="rec", name="rec")
                    nc.vector.reciprocal(rec[:], yv[:, :, HD:HW].squeeze(2))
                    for qb in range(4):
                        nc.vector.tensor_scalar_mul(
                            y2[:, qb * QB + h * HD:qb * QB + (h + 1) * HD],
                            yps2[u][:, qb * HW:qb * HW + HD],
                            rec[:, qb:qb + 1])
                es[hp] = []

            # ---- y2 [q, c] -> y^T [c, q] ----
            for i in range(4):
                tp = qq.tile([P, QB], bf16, tag="TP", name="ytp")
                for qb in range(4):
                    nc.tensor.transpose(
                        tp[:, qb * P:(qb + 1) * P],
                        y2[:, qb * QB + i * P:qb * QB + (i + 1) * P],
                        ident,
                    )
                nc.vector.tensor_copy(ysb[:, i * QB:(i + 1) * QB], tp[:])

            # ---- phase C: out^T = Wp @ y^T + bp ----
            for i in range(4):
                ops = qq.tile([P, 2 * QB], f32, tag="S", name="ops")
                for k in range(4):
                    nc.tensor.matmul(
                        ops[:, 0:QB],
                        wpt[:, k * C + i * P:k * C + (i + 1) * P],
                        ysb[:, k * QB:(k + 1) * QB],
                        start=(k == 0), stop=(k == 3),
                    )
                ot = sp.tile([P, QB], f32, tag="ot", name="ot")
                nc.vector.tensor_scalar_add(ot[:], ops[:, 0:QB],
                                            vecs[:, 36 + i:37 + i])
                nc.sync.dma_start(out=outT[i * P:(i + 1) * P, :], in_=ot[:])

    _split_multi_waits(nc, mybir)
    return nc


def _get_nc(with_bias=False):
    if "nc" not in _cache:
        _cache["nc"] = _build_nc()
    return _cache["nc"]


def _make_in_maps(x, weight, Wv, bv, Wp, bp, state):
    x = np.asarray(x, np.float32)
    w = np.asarray(weight, np.float32)[:, :, 0]
    if not int(np.asarray(state)):
        w = np.ones_like(w)
    WvT = np.ascontiguousarray(np.asarray(Wv, np.float32).T)
    WpT = np.ascontiguousarray(np.asarray(Wp, np.float32).T)
    bv = np.asarray(bv, np.float32)
    bp = np.asarray(bp, np.float32)
    scale = 1.0 / np.sqrt(HD)

    # [c-row-block, 128, cols] layouts for Wv^T / Wp^T
    wvt4 = WvT.reshape(4, P, C).transpose(1, 0, 2).reshape(P, 4 * C)
    wpt4 = WpT.reshape(4, P, C).transpose(1, 0, 2).reshape(P, 4 * C)
    wvt4 = np.ascontiguousarray(wvt4).astype(ml_dtypes.bfloat16)
    wpt4 = np.ascontiguousarray(wpt4).astype(ml_dtypes.bfloat16)

    in_maps = []
    for core in range(8):
        b, p = core // 4, core % 4
        nreal = QB * (p + 1)
        perm = np.concatenate(
            [np.arange(QB * p, QB * (p + 1)), np.arange(0, QB * p)])
        kx = np.zeros((T, C), np.float32)
        kx[:nreal] = x[b][perm]
        wp_ = np.zeros((T,), np.float32)
        wp_[:nreal] = w[b][perm]
        kxT = np.ascontiguousarray(kx.T)  # [C, T]
        kxd = np.ascontiguousarray(
            kxT.reshape(4, P, T).transpose(1, 0, 2)).astype(ml_dtypes.bfloat16)

        vecs = np.zeros((P, 40), np.float32)
        vecs[:, 0:NSB] = wp_.reshape(NSB, P).T
        vecs[:, NSB:2 * NSB] = (wp_ * scale).reshape(NSB, P).T
        vecs[:, 32:36] = bv.reshape(4, P).T
        vecs[:, 36:40] = bp.reshape(4, P).T

        real = (np.arange(T) < nreal).astype(np.float32)  # [T]
        realz = np.repeat(
            real.reshape(NSB, P).T[:, :, None], NH, axis=2).reshape(P, P)

        in_maps.append({
            "kxd": kxd.reshape(P, 4, T),
            "wvtd": wvt4,
            "wptd": wpt4,
            "vecsd": vecs,
            "realzd": realz.astype(ml_dtypes.bfloat16),
        })
    return in_maps


def _gather(results, x):
    out = np.empty((B, T, C), np.float32)
    for core in range(8):
        b, p = core // 4, core % 4
        out[b, QB * p:QB * (p + 1), :] = results[core]["outT"].T
    return out


def _run(in_maps, with_bias=False, **kw):
    from concourse.bass_utils import run_bass_kernel_spmd
    return run_bass_kernel_spmd(
        _get_nc(), in_maps, list(range(8)), **kw)


def kernel(x, weight, Wv, bv, Wp, bp, state):
    in_maps = _make_in_maps(x, weight, Wv, bv, Wp, bp, state)
    res = _run(in_maps)
    return _gather(res.results, x)


# revision 11
# speedup vs baseline: 1.3295x; 1.0318x over previous
"""Causal self-attention (weight-modulated) Trainium2 kernel, 8-core SPMD.

Reference semantics (B=2, T=2048, C=512, 8 heads, hd=64):
    v0  = x @ Wv.T + bv
    v   = v0 * w[:, :, None]            # w = weight[:, :, 0]
    att = softmax(mask((v0h @ v0h^T) * w[key] / sqrt(hd)))
    y   = att @ vh
    out = y @ Wp.T + bp

Sharding: core = (b, p) with b = batch, p = query-quarter. Each core
computes 512 contiguous query rows against all keys of its batch.
Keys are host-permuted so the causal diagonal 512-block sits at key
slot 0 for every core; the program is identical across cores (SPMD).

Dataflow per core:
  A: vT = (x @ Wv.T + bv)^T            [c, keyslot]  (bf16, 4x matmul)
  T: va = transpose(vT) * w[key]       [keyslot, (h: v_h | real)]
     via PE transposes (cost = 128 rows each; much cheaper than a
     second matmul pass).  "real" = 1 for real keys, 0 for padding:
     it feeds the softmax-denominator column of the AV matmul, so no
     -inf bias is needed for padding (padded x columns are zero).
  B (per head pair): scores = vT^T vT into PSUM [keyslot, 2*512],
     trimmed causally on the diagonal 512-block; e = exp(w8 * s) on
     ACT; in-diagonal triangular mask applied post-exp as a 0/1
     multiply on DVE (cheap 2-byte op); AV with e as the stationary
     operand: yps[q, h: y_h | denom] += e_tile^T @ [va_h | real]
     (N=65 per matmul -- half the streamed rows of the [hd,q] form),
     then normalize with DVE reciprocal + per-partition scalars.
  C: y2 [q, c] -> PE transpose -> y^T [c, q]; out^T = Wp @ y^T + bp.

Phase A/T work is interleaved into head-pair 0's slot loop so the
ACT engine gets exp work early and the PE never idles (p-state).
"""

import ml_dtypes
import numpy as np

B, T, C = 2, 2048, 512
NH, HD = 8, 64
P = 128
QB = 512                # query rows per core
NSB = 16                # key sub-blocks of 128
HW = HD + 1             # head window in va / yps: 64 values + denom
VAW = NH * HW           # 520 columns per key sub-block in va

_cache = {}


def _split_multi_waits(nc, mybir):
    """Walrus in this container encodes at most ONE sync wait (and one
    update) per instruction; Tile's sem assignment emits several. Hoist
    excess waits onto single-wait NOPs placed just before the
    instruction on the same engine (sequencer semantics are identical:
    the engine blocks on each wait, then issues the instruction), and
    excess updates of non-DMA instructions onto NOPs just after."""
    dma_ops = {"DMACopy", "DMATranspose", "TensorCopy"}
    for f in nc.m.functions:
        for bb in f.blocks:
            new = []
            changed = False
            for inst in bb.instructions:
                si = inst.sync_info
                waits = list(si.on_wait or []) if si is not None else []
                ups = list(si.on_update or []) if si is not None else []
                is_dma = inst.concise_opcode() in dma_ops if hasattr(
                    inst, "concise_opcode") else False
                post = []
                if si is not None and len(waits) > 1:
                    for w in waits[:-1]:
                        nop = mybir.InstNoOp(
                            name=nc.get_next_instruction_name(),
                            sync_info=mybir.SyncInfo(on_wait=[w], on_update=[]),
                            bass_nofuse=True,
                            engine=inst.engine,
                        )
                        nc.register_instruction(nop, overwrite=True)
                        new.append(nop)
                    waits = waits[-1:]
                    inst.sync_info = mybir.SyncInfo(on_wait=waits, on_update=ups)
                    changed = True
                if si is not None and len(ups) > 1 and not is_dma:
                    for u in ups[1:]:
                        nop = mybir.InstNoOp(
                            name=nc.get_next_instruction_name(),
                            sync_info=mybir.SyncInfo(on_wait=[], on_update=[u]),
                            bass_nofuse=True,
                            engine=inst.engine,
                        )
                        nc.register_instruction(nop, overwrite=True)
                        post.append(nop)
                    inst.sync_info = mybir.SyncInfo(
                        on_wait=waits, on_update=ups[:1])
                    changed = True
                new.append(inst)
                new.extend(post)
            if changed:
                bb.instructions = new


def _ctri_const():
    # [identity(128) | tri01(128)]: identity feeds PE transposes; tri01
    # is the in-diagonal causal mask: tri01[s, t] = 1 iff t >= s.
    s = np.arange(P)[:, None]
    t = np.arange(P)[None, :]
    out = np.empty((P, 2 * P), np.float32)
    out[:, 0:P] = (t == s)
    out[:, P:2 * P] = (t >= s)
    return out.astype(ml_dtypes.bfloat16)


def _build_nc():
    import concourse.bass as bass
    import concourse.mybir as mybir

    from concourse.tile import TileContext
    f32 = mybir.dt.float32
    bf16 = mybir.dt.bfloat16
    AF = mybir.ActivationFunctionType

    nc = bass.Bass()

    # vecs columns: 0-15 w (per key sub-block), 16-31 w/sqrt(hd),
    # 32-35 bv (c-block major), 36-39 bp
    kxd = nc.dram_tensor("kxd", [P, 4, T], bf16, kind="ExternalInput")
    wvtd = nc.dram_tensor("wvtd", [P, 4 * C], bf16, kind="ExternalInput")
    wptd = nc.dram_tensor("wptd", [P, 4 * C], bf16, kind="ExternalInput")
    vecsd = nc.dram_tensor("vecsd", [P, 40], f32, kind="ExternalInput")
    realzd = nc.dram_tensor("realzd", [P, P], bf16, kind="ExternalInput")
    outT = nc.dram_tensor("outT", [C, QB], f32, kind="ExternalOutput")

    ctri_d = nc.inline_tensor(_ctri_const(), name="ctri")

    with TileContext(nc) as tc:
        with (
            tc.tile_pool(name="persist", bufs=1) as pp,
            tc.tile_pool(name="stream", bufs=3) as sp,
            tc.tile_pool(name="psum", bufs=2, space="PSUM") as qq,
        ):
            # ---- persistent SBUF tensors ----
            kx = pp.tile([P, 4 * T], bf16, tag="kx")        # x^T, c-blk major
            wvt = pp.tile([P, 4 * C], bf16, tag="wvt")      # Wv^T, row-blk major
            wpt = pp.tile([P, 4 * C], bf16, tag="wpt")
            vT = pp.tile([P, 4 * T], bf16, tag="vT")        # v0^T, c-blk major
            va = pp.tile([P, NSB * VAW], bf16, tag="va")    # [slot, h: v|real]
            y2 = pp.tile([P, 4 * QB], bf16, tag="y2")       # [q, qb-major c]
            ysb = pp.tile([P, 4 * QB], bf16, tag="ysb")     # y^T, c-blk major
            vecs = pp.tile([P, 40], f32, tag="vecs")
            realz = pp.tile([P, P], bf16, tag="realz")
            ctri = pp.tile([P, 2 * P], bf16, tag="ctri")
            ident = ctri[:, 0:P]
            tri01 = ctri[:, P:2 * P]

            kx3 = kx[:].rearrange("p (k t) -> p k t", t=T)
            nc.sync.dma_start(out=wvt[:], in_=wvtd[:])
            nc.sync.dma_start(out=kx3[:, :, 0:QB], in_=kxd[:, :, 0:QB])
            nc.sync.dma_start(out=kx3[:, :, QB:2 * QB], in_=kxd[:, :, QB:2 * QB])
            nc.sync.dma_start(out=ctri[:], in_=ctri_d[:])
            nc.sync.dma_start(out=vecs[:], in_=vecsd[:])
            nc.sync.dma_start(out=realz[:], in_=realzd[:])
            nc.sync.dma_start(out=kx3[:, :, 2 * QB:3 * QB],
                              in_=kxd[:, :, 2 * QB:3 * QB])
            nc.sync.dma_start(out=kx3[:, :, 3 * QB:4 * QB],
                              in_=kxd[:, :, 3 * QB:4 * QB])
            nc.sync.dma_start(out=wpt[:], in_=wptd[:])

            # denominator indicator column: va[s, sb, h, 64] = real(sb,s)
            va4 = va[:].rearrange("p (s h w) -> p s h w", h=NH, w=HW)
            nc.vector.tensor_copy(
                va4[:, :, :, HD:HW].squeeze(3),
                realz[:].rearrange("p (s h) -> p s h", h=NH),
            )

            def emit_A(j, i):
                # vT[c-block i, key slots j*512:(j+1)*512]
                ps = qq.tile([P, QB], f32, tag="A", name="vps", bufs=1)
                for k in range(4):
                    nc.tensor.matmul(
                        ps[:],
                        wvt[:, k * C + i * P:k * C + (i + 1) * P],
                        kx[:, k * T + j * QB:k * T + (j + 1) * QB],
                        start=(k == 0), stop=(k == 3),
                    )
                nc.vector.tensor_scalar_add(
                    vT[:, i * T + j * QB:i * T + (j + 1) * QB],
                    ps[:], vecs[:, 32 + i:33 + i])

            def emit_T(sb):
                # va[sb] = transpose(vT[:, sb]) * w[key]
                tp = qq.tile([P, QB], bf16, tag="TP", name="tp", bufs=1)
                for i in range(4):
                    nc.tensor.transpose(
                        tp[:, i * P:(i + 1) * P],
                        vT[:, i * T + sb * P:i * T + (sb + 1) * P],
                        ident,
                    )
                nc.vector.tensor_scalar_mul(
                    va4[:, sb:sb + 1, :, 0:HD].squeeze(1),
                    tp[:].rearrange("p (h d) -> p h d", d=HD),
                    vecs[:, sb:sb + 1],
                )

            # A(0,0) computes the head-pair-0 vT block for keys/queries
            # 0..511, which is all QKE(0) slots 0-3 need -- everything else
            # drains 2-items-per-slot inside the pair-0 QKE loop so exp work
            # reaches the ACT engine as early as possible.
            work = []
            for i in range(1, 4):
                work.append((emit_A, 0, i))
            for sb in range(4):
                work.append((emit_T, sb))
            for j in range(1, 4):
                for i in range(4):
                    work.append((emit_A, j, i))
                for sb in range(4 * j, 4 * j + 4):
                    work.append((emit_T, sb))

            emit_A(0, 0)

            # ---- phase B: head pairs, software-pipelined ----
            # QKE(hp) slot sb: scores for 128 keys x all later queries of
            # both heads, exp (per-key scale folds w and 1/sqrt(hd)),
            # 0/1 triangular mask on the diagonal square.  The 16 e tiles
            # of a pair stay alive (bufs=33) so AV can then run qb-major
            # with one complete PSUM accumulation group at a time (the PE
            # model corrupts interleaved open groups within a bank).
            es = [[] for _ in range(NH // 2)]

            def qke_slot(hp, sb):
                off = sb * P if sb < 4 else 0
                spair = qq.tile([P, 2 * QB], f32, tag="S", name="spair")
                sp3 = spair[:].rearrange("p (u t) -> p u t", t=QB)
                for u in range(2):
                    po = u * HD
                    nc.tensor.matmul(
                        sp3[:, u, off:QB],
                        vT[po:po + HD,
                           hp * T + sb * P:hp * T + (sb + 1) * P],
                        vT[po:po + HD, hp * T + off:hp * T + QB],
                        start=True, stop=True,
                    )
                e = sp.tile([P, 2 * QB], bf16, tag="e", name="e", bufs=33)
                e3 = e[:].rearrange("p (u t) -> p u t", t=QB)
                nc.scalar.activation(
                    e3[:, :, off:QB], sp3[:, :, off:QB], AF.Exp,
                    scale=vecs[:, 16 + sb:17 + sb])
                if sb < 4:
                    for u in range(2):
                        seg = e[:, u * QB + off:u * QB + off + P]
                        nc.vector.tensor_mul(seg, seg, tri01)
                es[hp].append(e)

            def av_group(hp, yps2, u, qb):
                h = 2 * hp + u
                sbs = [s for s in range(NSB) if s >= 4 or s <= qb]
                for n, sb in enumerate(sbs):
                    nc.tensor.matmul(
                        yps2[u][:, qb * HW:(qb + 1) * HW],
                        es[hp][sb][:, u * QB + qb * P:u * QB + (qb + 1) * P],
                        va[:, sb * VAW + h * HW:sb * VAW + (h + 1) * HW],
                        start=(n == 0), stop=(n == len(sbs) - 1),
                    )

            for sb in range(NSB):
                for _ in range(2):
                    if work:
                        fn, *args = work.pop(0)
                        fn(*args)
                qke_slot(0, sb)

            for hp in range(NH // 2):
                yps2 = [qq.tile([P, 4 * HW], f32, tag=f"Y{u}", name=f"yps{u}",
                                bufs=1) for u in range(2)]
                gi = 0
                for u in range(2):
                    for qb in range(4):
                        av_group(hp, yps2, u, qb)
                        if hp + 1 < NH // 2:
                            qke_slot(hp + 1, 2 * gi)
                            qke_slot(hp + 1, 2 * gi + 1)
                        gi += 1
                for u in range(2):
                    h = 2 * hp + u
                    yv = yps2[u][:].rearrange("p (q w) -> p q w", w=HW)
                    rec = sp.tile([P, 4], f32, tag="rec", name="rec")
                    nc.vector.reciprocal(rec[:], yv[:, :, HD:HW].squeeze(2))
                    for qb in range(4):
                        nc.vector.tensor_scalar_mul(
                            y2[:, qb * QB + h * HD:qb * QB + (h + 1) * HD],
                            yps2[u][:, qb * HW:qb * HW + HD],
                            rec[:, qb:qb + 1])
                es[hp] = []

            # ---- y2 [q, c] -> y^T [c, q] ----
            for i in range(4):
                tp = qq.tile([P, QB], bf16, tag="TP", name="ytp", bufs=1)
                for qb in range(4):
                    nc.tensor.transpose(
                        tp[:, qb * P:(qb + 1) * P],
                        y2[:, qb * QB + i * P:qb * QB + (i + 1) * P],
                        ident,
                    )
                nc.vector.tensor_copy(ysb[:, i * QB:(i + 1) * QB], tp[:])

            # ---- phase C: out^T = Wp @ y^T + bp ----
            for i in range(4):
                ops = qq.tile([P, 2 * QB], f32, tag="S", name="ops")
                for k in range(4):
                    nc.tensor.matmul(
                        ops[:, 0:QB],
                        wpt[:, k * C + i * P:k * C + (i + 1) * P],
                        ysb[:, k * QB:(k + 1) * QB],
                        start=(k == 0), stop=(k == 3),
                    )
                ot = sp.tile([P, QB], f32, tag="ot", name="ot")
                nc.vector.tensor_scalar_add(ot[:], ops[:, 0:QB],
                                            vecs[:, 36 + i:37 + i])
                nc.sync.dma_start(out=outT[i * P:(i + 1) * P, :], in_=ot[:])

    _split_multi_waits(nc, mybir)
    return nc


def _get_nc(with_bias=False):
    if "nc" not in _cache:
        _cache["nc"] = _build_nc()
    return _cache["nc"]


def _make_in_maps(x, weight, Wv, bv, Wp, bp, state):
    x = np.asarray(x, np.float32)
    w = np.asarray(weight, np.float32)[:, :, 0]
    if not int(np.asarray(state)):
        w = np.ones_like(w)
    WvT = np.ascontiguousarray(np.asarray(Wv, np.float32).T)
    WpT = np.ascontiguousarray(np.asarray(Wp, np.float32).T)
    bv = np.asarray(bv, np.float32)
    bp = np.asarray(bp, np.float32)
    scale = 1.0 / np.sqrt(HD)

    # [c-row-block, 128, cols] layouts for Wv^T / Wp^T
    wvt4 = WvT.reshape(4, P, C).transpose(1, 0, 2).reshape(P, 4 * C)
    wpt4 = WpT.reshape(4, P, C).transpose(1, 0, 2).reshape(P, 4 * C)
    wvt4 = np.ascontiguousarray(wvt4).astype(ml_dtypes.bfloat16)
    wpt4 = np.ascontiguousarray(wpt4).astype(ml_dtypes.bfloat16)

    in_maps = []
    for core in range(8):
        b, p = core // 4, core % 4
        nreal = QB * (p + 1)
        perm = np.concatenate(
            [np.arange(QB * p, QB * (p + 1)), np.arange(0, QB * p)])
        kx = np.zeros((T, C), np.float32)
        kx[:nreal] = x[b][perm]
        wp_ = np.zeros((T,), np.float32)
        wp_[:nreal] = w[b][perm]
        kxT = np.ascontiguousarray(kx.T)  # [C, T]
        kxd = np.ascontiguousarray(
            kxT.reshape(4, P, T).transpose(1, 0, 2)).astype(ml_dtypes.bfloat16)

        vecs = np.zeros((P, 40), np.float32)
        vecs[:, 0:NSB] = wp_.reshape(NSB, P).T
        vecs[:, NSB:2 * NSB] = (wp_ * scale).reshape(NSB, P).T
        vecs[:, 32:36] = bv.reshape(4, P).T
        vecs[:, 36:40] = bp.reshape(4, P).T

        real = (np.arange(T) < nreal).astype(np.float32)  # [T]
        realz = np.repeat(
            real.reshape(NSB, P).T[:, :, None], NH, axis=2).reshape(P, P)

        in_maps.append({
            "kxd": kxd.reshape(P, 4, T),
            "wvtd": wvt4,
            "wptd": wpt4,
            "vecsd": vecs,
            "realzd": realz.astype(ml_dtypes.bfloat16),
        })
    return in_maps


def _gather(results, x):
    out = np.empty((B, T, C), np.float32)
    for core in range(8):
        b, p = core // 4, core % 4
        out[b, QB * p:QB * (p + 1), :] = results[core]["outT"].T
    return out


def _run(in_maps, with_bias=False, **kw):
    from concourse.bass_utils import run_bass_kernel_spmd
    return run_bass_kernel_spmd(
        _get_nc(), in_maps, list(range(8)), **kw)


def kernel(x, weight, Wv, bv, Wp, bp, state):
    in_maps = _make_in_maps(x, weight, Wv, bv, Wp, bp, state)
    res = _run(in_maps)
    return _gather(res.results, x)


# revision 13
# speedup vs baseline: 1.4269x; 1.0732x over previous
"""Causal self-attention (weight-modulated) Trainium2 kernel, 8-core SPMD.

Reference semantics (B=2, T=2048, C=512, 8 heads, hd=64):
    v0  = x @ Wv.T + bv
    v   = v0 * w[:, :, None]            # w = weight[:, :, 0]
    att = softmax(mask((v0h @ v0h^T) * w[key] / sqrt(hd)))
    y   = att @ vh
    out = y @ Wp.T + bp

Sharding: core = (b, p) with b = batch, p = query-quarter. Each core
computes 512 contiguous query rows against all keys of its batch.
Keys are host-permuted so the causal diagonal 512-block sits at key
slot 0 for every core; the program is identical across cores (SPMD).

Dataflow per core:
  A: vT = (x @ Wv.T + bv)^T            [c, keyslot]  (bf16, 4x matmul)
  T: va = transpose(vT) * w[key]       [keyslot, (h: v_h | real)]
     via PE transposes (cost = 128 rows each; much cheaper than a
     second matmul pass).  "real" = 1 for real keys, 0 for padding:
     it feeds the softmax-denominator column of the AV matmul, so no
     -inf bias is needed for padding (padded x columns are zero).
  B (per head pair): scores = vT^T vT into PSUM [keyslot, 2*512],
     trimmed causally on the diagonal 512-block; e = exp(w8 * s) on
     ACT; in-diagonal triangular mask applied post-exp as a 0/1
     multiply on DVE (cheap 2-byte op); AV with e as the stationary
     operand: yps[q, h: y_h | denom] += e_tile^T @ [va_h | real]
     (N=65 per matmul -- half the streamed rows of the [hd,q] form),
     then normalize with DVE reciprocal + per-partition scalars.
  C: y2 [q, c] -> PE transpose -> y^T [c, q]; out^T = Wp @ y^T + bp.

Phase A/T work is interleaved into head-pair 0's slot loop so the
ACT engine gets exp work early and the PE never idles (p-state).
"""

import ml_dtypes
import numpy as np

B, T, C = 2, 2048, 512
NH, HD = 8, 64
P = 128
QB = 512                # query rows per core
NSB = 16                # key sub-blocks of 128
HW = HD + 1             # head window in va / yps: 64 values + denom
VAW = NH * HW           # 520 columns per key sub-block in va

_cache = {}


def _split_multi_waits(nc, mybir):
    """Walrus in this container encodes at most ONE sync wait (and one
    update) per instruction; Tile's sem assignment emits several. Hoist
    excess waits onto single-wait NOPs placed just before the
    instruction on the same engine (sequencer semantics are identical:
    the engine blocks on each wait, then issues the instruction), and
    excess updates of non-DMA instructions onto NOPs just after."""
    dma_ops = {"DMACopy", "DMATranspose", "TensorCopy"}
    for f in nc.m.functions:
        for bb in f.blocks:
            new = []
            changed = False
            for inst in bb.instructions:
                si = inst.sync_info
                waits = list(si.on_wait or []) if si is not None else []
                ups = list(si.on_update or []) if si is not None else []
                is_dma = inst.concise_opcode() in dma_ops if hasattr(
                    inst, "concise_opcode") else False
                post = []
                if si is not None and len(waits) > 1:
                    for w in waits[:-1]:
                        nop = mybir.InstNoOp(
                            name=nc.get_next_instruction_name(),
                            sync_info=mybir.SyncInfo(on_wait=[w], on_update=[]),
                            bass_nofuse=True,
                            engine=inst.engine,
                        )
                        nc.register_instruction(nop, overwrite=True)
                        new.append(nop)
                    waits = waits[-1:]
                    inst.sync_info = mybir.SyncInfo(on_wait=waits, on_update=ups)
                    changed = True
                if si is not None and len(ups) > 1 and not is_dma:
                    for u in ups[1:]:
                        nop = mybir.InstNoOp(
                            name=nc.get_next_instruction_name(),
                            sync_info=mybir.SyncInfo(on_wait=[], on_update=[u]),
                            bass_nofuse=True,
                            engine=inst.engine,
                        )
                        nc.register_instruction(nop, overwrite=True)
                        post.append(nop)
                    inst.sync_info = mybir.SyncInfo(
                        on_wait=waits, on_update=ups[:1])
                    changed = True
                new.append(inst)
                new.extend(post)
            if changed:
                bb.instructions = new


def _ctri_const():
    # [identity(128) | tri01(128)]: identity feeds PE transposes; tri01
    # is the in-diagonal causal mask: tri01[s, t] = 1 iff t >= s.
    s = np.arange(P)[:, None]
    t = np.arange(P)[None, :]
    out = np.empty((P, 2 * P), np.float32)
    out[:, 0:P] = (t == s)
    out[:, P:2 * P] = (t >= s)
    return out.astype(ml_dtypes.bfloat16)


def _build_nc():
    import concourse.bass as bass
    import concourse.mybir as mybir

    from concourse.tile import TileContext
    f32 = mybir.dt.float32
    bf16 = mybir.dt.bfloat16
    AF = mybir.ActivationFunctionType

    nc = bass.Bass()

    # vecs columns: 0-15 w (per key sub-block), 16-31 w/sqrt(hd),
    # 32-35 bv (c-block major), 36-39 bp
    kxd = nc.dram_tensor("kxd", [P, 4, T], bf16, kind="ExternalInput")
    wvtd = nc.dram_tensor("wvtd", [P, 4 * C], bf16, kind="ExternalInput")
    wptd = nc.dram_tensor("wptd", [P, 4 * C], bf16, kind="ExternalInput")
    vecsd = nc.dram_tensor("vecsd", [P, 40], f32, kind="ExternalInput")
    realzd = nc.dram_tensor("realzd", [P, P], bf16, kind="ExternalInput")
    outT = nc.dram_tensor("outT", [C, QB], f32, kind="ExternalOutput")

    ctri_d = nc.inline_tensor(_ctri_const(), name="ctri")

    with TileContext(nc) as tc:
        with (
            tc.tile_pool(name="persist", bufs=1) as pp,
            tc.tile_pool(name="stream", bufs=3) as sp,
            tc.tile_pool(name="psum", bufs=2, space="PSUM") as qq,
        ):
            # ---- persistent SBUF tensors ----
            kx = pp.tile([P, 4 * T], bf16, tag="kx")        # x^T, c-blk major
            wvt = pp.tile([P, 4 * C], bf16, tag="wvt")      # Wv^T, row-blk major
            wpt = pp.tile([P, 4 * C], bf16, tag="wpt")
            vT = pp.tile([P, 4 * T], bf16, tag="vT")        # v0^T, c-blk major
            va = pp.tile([P, NSB * VAW], bf16, tag="va")    # [slot, h: v|real]
            y2 = pp.tile([P, 4 * QB], bf16, tag="y2")       # [q, qb-major c]
            ysb = pp.tile([P, 4 * QB], bf16, tag="ysb")     # y^T, c-blk major
            vecs = pp.tile([P, 40], f32, tag="vecs")
            realz = pp.tile([P, P], bf16, tag="realz")
            ctri = pp.tile([P, 2 * P], bf16, tag="ctri")
            ident = ctri[:, 0:P]
            tri01 = ctri[:, P:2 * P]

            # warm the ACT Exp table during the input-DMA wait
            warm = pp.tile([1, 2], f32, tag="warm")
            nc.vector.memset(warm[:], 0.0)
            nc.scalar.activation(warm[:, 1:2], warm[:, 0:1], AF.Exp)

            kx3 = kx[:].rearrange("p (k t) -> p k t", t=T)
            nc.sync.dma_start(out=wvt[:], in_=wvtd[:])
            nc.sync.dma_start(out=kx3[:, :, 0:QB], in_=kxd[:, :, 0:QB])
            nc.sync.dma_start(out=vecs[:], in_=vecsd[:])
            nc.sync.dma_start(out=ctri[:], in_=ctri_d[:])
            nc.sync.dma_start(out=realz[:], in_=realzd[:])
            nc.sync.dma_start(out=kx3[:, :, QB:2 * QB], in_=kxd[:, :, QB:2 * QB])
            nc.sync.dma_start(out=kx3[:, :, 2 * QB:3 * QB],
                              in_=kxd[:, :, 2 * QB:3 * QB])
            nc.sync.dma_start(out=kx3[:, :, 3 * QB:4 * QB],
                              in_=kxd[:, :, 3 * QB:4 * QB])
            nc.sync.dma_start(out=wpt[:], in_=wptd[:])

            # denominator indicator column: va[s, sb, h, 64] = real(sb,s)
            va4 = va[:].rearrange("p (s h w) -> p s h w", h=NH, w=HW)
            nc.vector.tensor_copy(
                va4[:, :, :, HD:HW].squeeze(3),
                realz[:].rearrange("p (s h) -> p s h", h=NH),
            )

            def emit_A(j, i):
                # vT[c-block i, key slots j*512:(j+1)*512]
                ps = qq.tile([P, QB], f32, tag="A", name="vps", bufs=1)
                for k in range(4):
                    nc.tensor.matmul(
                        ps[:],
                        wvt[:, k * C + i * P:k * C + (i + 1) * P],
                        kx[:, k * T + j * QB:k * T + (j + 1) * QB],
                        start=(k == 0), stop=(k == 3),
                    )
                nc.vector.tensor_scalar_add(
                    vT[:, i * T + j * QB:i * T + (j + 1) * QB],
                    ps[:], vecs[:, 32 + i:33 + i])

            def emit_T(sb):
                # va[sb] = transpose(vT[:, sb]) * w[key]
                tp = qq.tile([P, QB], bf16, tag="TP", name="tp", bufs=1)
                for i in range(4):
                    nc.tensor.transpose(
                        tp[:, i * P:(i + 1) * P],
                        vT[:, i * T + sb * P:i * T + (sb + 1) * P],
                        ident,
                    )
                nc.vector.tensor_scalar_mul(
                    va4[:, sb:sb + 1, :, 0:HD].squeeze(1),
                    tp[:].rearrange("p (h d) -> p h d", d=HD),
                    vecs[:, sb:sb + 1],
                )

            # A(0,0) computes the head-pair-0 vT block for keys/queries
            # 0..511, which is all QKE(0) slots 0-3 need -- everything else
            # drains 2-items-per-slot inside the pair-0 QKE loop so exp work
            # reaches the ACT engine as early as possible.  i-minor j-major
            # order: pair-0 slot sb is gated only by A(sb//4, i=0), so the
            # i=0 column goes first; transposes (gating only the AV groups
            # after slot 15) go last.
            work = []
            for j in range(1, 4):
                work.append((emit_A, j, 0))
            for i in range(1, 4):
                for j in range(4):
                    work.append((emit_A, j, i))
            for sb in range(NSB):
                work.append((emit_T, sb))

            emit_A(0, 0)

            # ---- phase B: head pairs, software-pipelined ----
            # QKE(hp) slot sb: scores for 128 keys x all later queries of
            # both heads, exp (per-key scale folds w and 1/sqrt(hd)),
            # 0/1 triangular mask on the diagonal square.  The 16 e tiles
            # of a pair stay alive (bufs=33) so AV can then run qb-major
            # with one complete PSUM accumulation group at a time (the PE
            # model corrupts interleaved open groups within a bank).
            es = [[] for _ in range(NH // 2)]

            def qke_slot(hp, sb):
                off = sb * P if sb < 4 else 0
                spair = qq.tile([P, 2 * QB], f32, tag="S", name="spair")
                sp3 = spair[:].rearrange("p (u t) -> p u t", t=QB)
                for u in range(2):
                    po = u * HD
                    nc.tensor.matmul(
                        sp3[:, u, off:QB],
                        vT[po:po + HD,
                           hp * T + sb * P:hp * T + (sb + 1) * P],
                        vT[po:po + HD, hp * T + off:hp * T + QB],
                        start=True, stop=True,
                    )
                e = sp.tile([P, 2 * QB], bf16, tag="e", name="e", bufs=33)
                e3 = e[:].rearrange("p (u t) -> p u t", t=QB)
                nc.scalar.activation(
                    e3[:, :, off:QB], sp3[:, :, off:QB], AF.Exp,
                    scale=vecs[:, 16 + sb:17 + sb])
                if sb < 4:
                    for u in range(2):
                        seg = e[:, u * QB + off:u * QB + off + P]
                        nc.vector.tensor_mul(seg, seg, tri01)
                es[hp].append(e)

            def av_group(hp, yps2, u, qb):
                h = 2 * hp + u
                sbs = [s for s in range(NSB) if s >= 4 or s <= qb]
                for n, sb in enumerate(sbs):
                    nc.tensor.matmul(
                        yps2[u][:, qb * HW:(qb + 1) * HW],
                        es[hp][sb][:, u * QB + qb * P:u * QB + (qb + 1) * P],
                        va[:, sb * VAW + h * HW:sb * VAW + (h + 1) * HW],
                        start=(n == 0), stop=(n == len(sbs) - 1),
                    )

            for sb in range(NSB):
                for _ in range(2):
                    if work:
                        fn, *args = work.pop(0)
                        fn(*args)
                qke_slot(0, sb)

            for hp in range(NH // 2):
                yps2 = [qq.tile([P, 4 * HW], f32, tag=f"Y{u}", name=f"yps{u}",
                                bufs=1) for u in range(2)]
                gi = 0
                for u in range(2):
                    for qb in range(4):
                        av_group(hp, yps2, u, qb)
                        if hp + 1 < NH // 2:
                            qke_slot(hp + 1, 2 * gi)
                            qke_slot(hp + 1, 2 * gi + 1)
                        gi += 1
                for u in range(2):
                    h = 2 * hp + u
                    yv = yps2[u][:].rearrange("p (q w) -> p q w", w=HW)
                    rec = sp.tile([P, 4], f32, tag="rec", name="rec")
                    nc.vector.reciprocal(rec[:], yv[:, :, HD:HW].squeeze(2))
                    for qb in range(4):
                        nc.vector.tensor_scalar_mul(
                            y2[:, qb * QB + h * HD:qb * QB + (h + 1) * HD],
                            yps2[u][:, qb * HW:qb * HW + HD],
                            rec[:, qb:qb + 1])
                es[hp] = []

            # ---- y2 [q, c] -> y^T [c, q] ----
            for i in range(4):
                tp = qq.tile([P, QB], bf16, tag="TP", name="ytp", bufs=1)
                for qb in range(4):
                    nc.tensor.transpose(
                        tp[:, qb * P:(qb + 1) * P],
                        y2[:, qb * QB + i * P:qb * QB + (i + 1) * P],
                        ident,
                    )
                nc.vector.tensor_copy(ysb[:, i * QB:(i + 1) * QB], tp[:])

            # ---- phase C: out^T = Wp @ y^T + bp ----
            for i in range(4):
                ops = qq.tile([P, 2 * QB], f32, tag="S", name="ops")
                for k in range(4):
                    nc.tensor.matmul(
                        ops[:, 0:QB],
                        wpt[:, k * C + i * P:k * C + (i + 1) * P],
                        ysb[:, k * QB:(k + 1) * QB],
                        start=(k == 0), stop=(k == 3),
                    )
                ot = sp.tile([P, QB], f32, tag="ot", name="ot")
                nc.vector.tensor_scalar_add(ot[:], ops[:, 0:QB],
                                            vecs[:, 36 + i:37 + i])
                nc.sync.dma_start(out=outT[i * P:(i + 1) * P, :], in_=ot[:])

    _split_multi_waits(nc, mybir)
    return nc


def _get_nc(with_bias=False):
    if "nc" not in _cache:
        _cache["nc"] = _build_nc()
    return _cache["nc"]


def _make_in_maps(x, weight, Wv, bv, Wp, bp, state):
    x = np.asarray(x, np.float32)
    w = np.asarray(weight, np.float32)[:, :, 0]
    if not int(np.asarray(state)):
        w = np.ones_like(w)
    WvT = np.ascontiguousarray(np.asarray(Wv, np.float32).T)
    WpT = np.ascontiguousarray(np.asarray(Wp, np.float32).T)
    bv = np.asarray(bv, np.float32)
    bp = np.asarray(bp, np.float32)
    scale = 1.0 / np.sqrt(HD)

    # [c-row-block, 128, cols] layouts for Wv^T / Wp^T
    wvt4 = WvT.reshape(4, P, C).transpose(1, 0, 2).reshape(P, 4 * C)
    wpt4 = WpT.reshape(4, P, C).transpose(1, 0, 2).reshape(P, 4 * C)
    wvt4 = np.ascontiguousarray(wvt4).astype(ml_dtypes.bfloat16)
    wpt4 = np.ascontiguousarray(wpt4).astype(ml_dtypes.bfloat16)

    in_maps = []
    for core in range(8):
        b, p = core // 4, core % 4
        nreal = QB * (p + 1)
        perm = np.concatenate(
            [np.arange(QB * p, QB * (p + 1)), np.arange(0, QB * p)])
        kx = np.zeros((T, C), np.float32)
        kx[:nreal] = x[b][perm]
        wp_ = np.zeros((T,), np.float32)
        wp_[:nreal] = w[b][perm]
        kxT = np.ascontiguousarray(kx.T)  # [C, T]
        kxd = np.ascontiguousarray(
            kxT.reshape(4, P, T).transpose(1, 0, 2)).astype(ml_dtypes.bfloat16)

        vecs = np.zeros((P, 40), np.float32)
        vecs[:, 0:NSB] = wp_.reshape(NSB, P).T
        vecs[:, NSB:2 * NSB] = (wp_ * scale).reshape(NSB, P).T
        vecs[:, 32:36] = bv.reshape(4, P).T
        vecs[:, 36:40] = bp.reshape(4, P).T

        real = (np.arange(T) < nreal).astype(np.float32)  # [T]
        realz = np.repeat(
            real.reshape(NSB, P).T[:, :, None], NH, axis=2).reshape(P, P)

        in_maps.append({
            "kxd": kxd.reshape(P, 4, T),
            "wvtd": wvt4,
            "wptd": wpt4,
            "vecsd": vecs,
            "realzd": realz.astype(ml_dtypes.bfloat16),
        })
    return in_maps


def _gather(results, x):
    out = np.empty((B, T, C), np.float32)
    for core in range(8):
        b, p = core // 4, core % 4
        out[b, QB * p:QB * (p + 1), :] = results[core]["outT"].T
    return out


def _run(in_maps, with_bias=False, **kw):
    from concourse.bass_utils import run_bass_kernel_spmd
    return run_bass_kernel_spmd(
        _get_nc(), in_maps, list(range(8)), **kw)


def kernel(x, weight, Wv, bv, Wp, bp, state):
    in_maps = _make_in_maps(x, weight, Wv, bv, Wp, bp, state)
    res = _run(in_maps)
    return _gather(res.results, x)
